# revision 1
# baseline (speedup 1.0000x reference)
"""Trainium2 Bass kernel for the moe_routing classifier problem.

Computation (per batch row b, class c):
  cos[b,c,s]  = cosine(emb[b], weight[c,s])            (64 sub-prototypes)
  top-8 over s, softmax weights w, protos = sum_k w_k * weight[c, idx_k]
  out[b,c]    = ((1 + cosine(protos, emb[b])) / 2 + 1e-8) / 0.1

Key algebra used by the kernel (avoids gathers entirely):
  E[b,c,s]   = exp(score) masked to the top-8 entries (unnormalized softmax)
  dot2*Z     = sum_s E * dot_raw                        (Z cancels later)
  |protos|^2*Z^2 = E^T (W W^T) E  via per-class Gram matrices
  cos2       = (sum_s E*dot_raw) * inv|emb| / sqrt(E^T G E)

Sharding: classes are split across the 8 cores (32 classes each); emb is
replicated. Each core writes a [1024, 32] slice of the output.

Engine schedule: per batch tile, stage A (matmul -> exp -> top-8 mask) and
stage B (pair-transpose E -> EG matmul -> reductions) are emitted with a
one-tile skew so each engine's in-order stream never stalls on the
cross-engine chain of the same tile.
"""

import numpy as np

B, D, C, S = 1024, 128, 256, 64
NCORES = 8
C_LOC = C // NCORES        # 32 classes per core
CS = C_LOC * S             # 2048 anchor rows per core
P = 128                    # partitions
NBT = B // P               # 8 batch tiles
NWT = CS // P              # 16 weight tiles
EPS = 1e-8
SC_BIAS = 0.5 + EPS        # score = 0.5*cos + SC_BIAS
OUT_SCALE = 5.0            # ((1+x)/2 + 1e-8) / 0.1 = 5x + 5 + 1e-7
OUT_BIAS = 5.0 + 1e-7

_CACHE = {}


def build_nc():
    import concourse.bass as bass
    import concourse.tile as tile
    from concourse import bacc, mybir
    from concourse.masks import make_identity
    from contextlib import ExitStack

    f32 = mybir.dt.float32
    AF = mybir.ActivationFunctionType
    ALU = mybir.AluOpType

    nc = bacc.Bacc(None, target_bir_lowering=False)
    emb_d = nc.dram_tensor("emb", [B, D], f32, kind="ExternalInput")
    w_d = nc.dram_tensor("weight", [CS, D], f32, kind="ExternalInput")
    out_d = nc.dram_tensor("out", [B, C_LOC], f32, kind="ExternalOutput")

    with tile.TileContext(nc) as tc, ExitStack() as ctx:
        sing = ctx.enter_context(tc.tile_pool(name="sing", bufs=1))
        dram = ctx.enter_context(tc.tile_pool(name="dram", bufs=1, space="DRAM"))
        work = ctx.enter_context(tc.tile_pool(name="work", bufs=3))
        small = ctx.enter_context(tc.tile_pool(name="small", bufs=4))
        jk = ctx.enter_context(tc.tile_pool(name="jk", bufs=8))
        fpool = ctx.enter_context(tc.tile_pool(name="fpool", bufs=2))
        ps_mm = ctx.enter_context(tc.tile_pool(name="ps_mm", bufs=2, space="PSUM"))
        ps_tr = ctx.enter_context(tc.tile_pool(name="ps_tr", bufs=3, space="PSUM"))
        ps_eg = ctx.enter_context(tc.tile_pool(name="ps_eg", bufs=3, space="PSUM"))

        ident = sing.tile([P, P], f32)
        make_identity(nc, ident[:])
        sbias = sing.tile([P, 1], f32)     # score bias as AP (Exp needs AP bias)
        nc.vector.memset(sbias[:], SC_BIAS)

        # prefetch the EXP activation table during otherwise-idle preproc
        texp = sing.tile([P, 1], f32)
        nc.scalar.activation(texp[:], sbias[:], AF.Exp)

        # ---------------- load inputs (emb first, separate DMA queues) ----
        En = sing.tile([P, NBT, D], f32)   # emb rows, tiled by 128
        nc.sync.dma_start(En[:], emb_d[:].rearrange("(t p) d -> p t d", p=P))
        Wn = sing.tile([P, NWT, D], f32)   # weight rows, tiled by 128
        nc.gpsimd.dma_start(Wn[:], w_d[:].rearrange("(t p) d -> p t d", p=P))

        # ---------------- norms (emb before weights) ----------------
        esq = sing.tile([P, NBT], f32)
        for t in range(NBT):
            j = jk.tile([P, D], f32, tag="jact")
            nc.scalar.activation(j[:], En[:, t], AF.Square,
                                 accum_out=esq[:, t : t + 1])
        ne = sing.tile([P, NBT], f32)      # ||emb||
        nc.scalar.activation(ne[:], esq[:], AF.Sqrt)
        ine = sing.tile([P, NBT], f32)     # 1/||emb||
        hine = sing.tile([P, NBT], f32)    # 0.5/||emb||
        nc.vector.reciprocal_approx_accurate(ine[:], ne[:], hine[:])
        nc.vector.tensor_scalar_mul(hine[:], ine[:], 0.5)
        nwsq = sing.tile([P, NWT], f32)    # ||w_row||^2, row-tiled layout
        for t in range(NWT):
            j = jk.tile([P, D], f32, tag="jact")
            nc.scalar.activation(j[:], Wn[:, t], AF.Square,
                                 accum_out=nwsq[:, t : t + 1])

        # nw broadcast rows: roundtrip through DRAM to reorder + partition-bcast
        scr = dram.tile([CS], f32)
        nc.sync.dma_start(scr[:].rearrange("(t p) -> p t", p=P), nwsq[:])
        scr_bc = bass.AP(
            tensor=scr[:].tensor, offset=scr[:].offset,
            ap=[[0, P]] + list(scr[:].ap),
        )
        NWB = sing.tile([P, CS], f32)      # ||w_row|| broadcast over partitions
        nc.sync.dma_start(NWB[:], scr_bc)
        nc.scalar.activation(NWB[:], NWB[:], AF.Sqrt)

        # ---------------- transposed operands ----------------
        # normalize anchor rows first (per-partition scale), then transpose,
        # so VT chunks become available early for the first batch tile.
        nw_row = sing.tile([P, NWT], f32)   # ||w_row||, row-tiled
        inw_row = sing.tile([P, NWT], f32)  # 1/||w_row||
        inw_scr = sing.tile([P, NWT], f32)
        nc.scalar.activation(nw_row[:], nwsq[:], AF.Sqrt)
        nc.vector.reciprocal_approx_accurate(inw_row[:], nw_row[:], inw_scr[:])

        embT = sing.tile([P, B], f32)      # emb^T [d, b]
        for t in range(NBT):
            pst = ps_tr.tile([P, 2 * P], f32, tag="tr")
            nc.tensor.transpose(pst[:, :P], En[:, t], ident[:])
            nc.scalar.copy(embT[:, t * P : (t + 1) * P], pst[:, :P])

        VT = sing.tile([P, CS], f32)       # normalized anchors transposed
        Vn = sing.tile([P, NWT, D], f32)
        for t in range(NWT):
            nc.vector.tensor_scalar_mul(Vn[:, t], Wn[:, t],
                                        inw_row[:, t : t + 1])
            pst = ps_tr.tile([P, 2 * P], f32, tag="tr")
            nc.tensor.transpose(pst[:, :P], Vn[:, t], ident[:])
            nc.scalar.copy(VT[:, t * P : (t + 1) * P], pst[:, :P])



        # persistent per-tile outputs for the batched tail
        d2zall = sing.tile([P, NBT, C_LOC], f32)
        np2zall = sing.tile([P, NBT, C_LOC], f32)

        def build_gram():
            # per-class raw Gram matrices G_c = W_c W_c^T [64, 64], packed
            # into block-diagonal pair matrices GP[:, q*128:(q+1)*128] =
            # [[G_2q, 0], [0, G_2q+1]] so one full-size (0,0)-quadrant matmul
            # computes EG for a transposed class pair (quadrant matmuls
            # interleaved with transposes crash the device).
            WT = sing.tile([P, CS], f32)   # raw W^T [d, cs] (gram only)
            for t in range(NWT):
                pst = ps_tr.tile([P, 2 * P], f32, tag="tr")
                nc.tensor.transpose(pst[:, :P], Wn[:, t], ident[:])
                nc.scalar.copy(WT[:, t * P : (t + 1) * P], pst[:, :P])
            Gtmp = sing.tile([S, CS], f32)
            for c in range(C_LOC):
                cs = slice(c * S, (c + 1) * S)
                psg = ps_tr.tile([P, 2 * P], f32, tag="tr")
                nc.tensor.matmul(psg[:S, :S], WT[:, cs], WT[:, cs])
                nc.scalar.copy(Gtmp[:, cs], psg[:S, :S])
            GP = sing.tile([P, CS], f32)
            nc.vector.memset(GP[:], 0.0)
            gt3 = Gtmp[:].rearrange("p (q j) -> p q j", j=2 * S)
            gp3 = GP[:].rearrange("p (q j) -> p q j", j=2 * S)
            nc.sync.dma_start(gp3[0:S, :, 0:S], gt3[:, :, 0:S])
            nc.sync.dma_start(gp3[S : 2 * S, :, S : 2 * S], gt3[:, :, S : 2 * S])
            return GP

        tiles = {}

        def stageA(bt):
            bsl = slice(bt * P, (bt + 1) * P)
            exps = work.tile([P, CS], f32, tag="exps", bufs=2)
            dotr = work.tile([P, CS], f32, tag="dotr", bufs=2)
            for j in range(CS // 512):
                js = slice(j * 512, (j + 1) * 512)
                dotn = ps_mm.tile([P, 512], f32, tag="mm")
                nc.tensor.matmul(dotn[:], embT[:, bsl], VT[:, js])
                nc.scalar.activation(
                    exps[:, js], dotn[:], AF.Exp,
                    bias=sbias[:], scale=hine[:, bt : bt + 1],
                )
                nc.vector.tensor_mul(dotr[:, js], dotn[:], NWB[:, js])

            # top-8 selection per class: R = exps with top8 zeroed
            R = work.tile([P, CS], f32, tag="R", bufs=2)
            for c in range(C_LOC):
                cs = slice(c * S, (c + 1) * S)
                mx8 = small.tile([P, 8], f32, tag="mx8")
                nc.vector.max(out=mx8[:], in_=exps[:, cs])
                nc.vector.match_replace(
                    out=R[:, cs], in_to_replace=mx8[:],
                    in_values=exps[:, cs], imm_value=0.0,
                )
            E = work.tile([P, CS], f32, tag="E", bufs=5)
            nc.gpsimd.tensor_sub(E[:], exps[:], R[:])
            prod_d = work.tile([P, CS], f32, tag="pd", bufs=3)
            nc.gpsimd.tensor_mul(prod_d[:], E[:], dotr[:])
            tiles[bt] = (E, prod_d)

        def stageB(bt, GP):
            E, prod_d = tiles.pop(bt)
            nc.vector.tensor_reduce(
                d2zall[:, bt], prod_d[:].rearrange("p (c s) -> p c s", c=C_LOC),
                axis=mybir.AxisListType.X, op=ALU.add)
            prod_n = work.tile([P, CS], f32, tag="prod_n", bufs=2)
            for q8 in range(CS // 512):
                qs8 = slice(q8 * 512, (q8 + 1) * 512)
                pse = ps_eg.tile([P, 512], f32, tag="eg")
                pst = ps_tr.tile([P, 512], f32, tag="tr")
                Fq = fpool.tile([P, 512], f32, tag="F")
                for h in range(4):
                    q = 4 * q8 + h
                    nc.tensor.transpose(
                        pst[:, h * 128 : (h + 1) * 128],
                        E[:, q * 128 : (q + 1) * 128], ident[:],
                    )
                nc.scalar.copy(Fq[:], pst[:])
                for h in range(4):
                    q = 4 * q8 + h
                    nc.tensor.matmul(
                        pse[:, h * 128 : (h + 1) * 128],
                        Fq[:, h * 128 : (h + 1) * 128],
                        GP[:, q * 128 : (q + 1) * 128],
                    )
                nc.vector.tensor_mul(prod_n[:, qs8], pse[:], E[:, qs8])
            nc.vector.tensor_reduce(
                np2zall[:, bt], prod_n[:].rearrange("p (c s) -> p c s", c=C_LOC),
                axis=mybir.AxisListType.X, op=ALU.add)

        # ---------------- software-pipelined main loop (skew 3) ----------
        stageA(0)
        stageA(1)
        GP = build_gram()
        stageA(2)
        stageA(3)
        for bt in range(4, NBT):
            stageB(bt - 4, GP)
            stageA(bt)
        for bt in range(NBT - 4, NBT):
            stageB(bt, GP)

        # ---------------- batched tail ----------------
        # cos2 = d2z * ine / sqrt(np2z);  out = 5*cos2 + 5 + 1e-7
        nps = sing.tile([P, NBT, C_LOC], f32)
        nc.scalar.activation(nps[:], np2zall[:], AF.Sqrt)
        rnp = sing.tile([P, NBT, C_LOC], f32)
        c2 = sing.tile([P, NBT, C_LOC], f32)
        nc.vector.reciprocal_approx_accurate(rnp[:], nps[:], c2[:])
        nc.vector.tensor_mul(c2[:], d2zall[:], rnp[:])
        ine_b = ine[:, :, None].to_broadcast([P, NBT, C_LOC])
        nc.vector.tensor_mul(c2[:], c2[:], ine_b)
        osb = sing.tile([P, NBT, C_LOC], f32)
        nc.vector.tensor_scalar(
            osb[:], c2[:], OUT_SCALE, OUT_BIAS, op0=ALU.mult, op1=ALU.add)
        nc.sync.dma_start(out_d[:].rearrange("(t p) c -> p t c", p=P), osb[:])

    nc.compile()
    return nc


def _get_nc():
    if "nc" not in _CACHE:
        _CACHE["nc"] = build_nc()
    return _CACHE["nc"]


def kernel(emb: np.ndarray, weight: np.ndarray) -> np.ndarray:
    from concourse.bass_utils import run_bass_kernel_spmd

    emb = np.ascontiguousarray(np.asarray(emb, dtype=np.float32))
    weight = np.ascontiguousarray(np.asarray(weight, dtype=np.float32))
    assert emb.shape == (B, D) and weight.shape == (C, S, D)

    nc = _get_nc()
    in_maps = [
        {
            "emb": emb,
            "weight": np.ascontiguousarray(
                weight[i * C_LOC : (i + 1) * C_LOC].reshape(CS, D)
            ),
        }
        for i in range(NCORES)
    ]
    res = run_bass_kernel_spmd(nc, in_maps, core_ids=list(range(NCORES)))
    return np.concatenate(
        [res.results[i]["out"] for i in range(NCORES)], axis=1
    )



# revision 8
# speedup vs baseline: 1.3999x; 1.3999x over previous
"""Trainium2 Bass kernel for the moe_routing classifier problem (v2).

Computation (per batch row b, class c):
  score[b,c,s] = (1 + cos(emb[b], W[c,s]))/2 + 1e-8     (S=64 sub-prototypes)
  top-8 over s, softmax weights w, protos = sum_k w_k * W[c, idx_k]
  out[b,c]     = ((1 + cos(protos, emb[b]))/2 + 1e-8) / 0.1

Algebra (Z = unnormalized softmax sum cancels in the cosine ratio):
  E[b,cs]   = exp(score) masked to the top-8 entries   (threshold mask)
  d2'[b,c]  = sum_s E * (W_s . emb_b/|emb_b|)          (prodD in s-major space)
  np2[b,c]  = |L_c^T E_c^T|^2 = E^T G E                (G = W W^T = L L^T, host chol)
  out       = 5 * d2' / sqrt(np2) + 5 + 1e-7           (1/sqrt via exp(-.5 ln))

Host prep (per core, weights-only): V^T (rows normalized), W^T, block-diag
pair Cholesky factors LP, a block-ones reduction matrix; all bf16.

Device per batch tile (128 rows):
  PE : dot = embT^T V       (b-major, for scores)
       dotT = W embT        (s-major, raw dots for d2')
       F = E^T (16 transposes), M = LP^T F, segmented s-sums of
       [M^2 | F*dotT] via 16 accumulating block-ones matmuls -> psum [32,2,128]
  ACT: exps = Exp(dot*hine+bias) bf16; F/Msq copies; tail ln/exp/out-copy
  DVE: 32x MAX8 (8th-largest per class), E = mask*exps (bf16 2x), tail muls
  POOL: mask = exps >= t8 (broadcast threshold), prodD = dotT * F

Engines never touch Sqrt: all ACT funcs (Exp/Ln/Square/Copy/Identity) live in
the natural_log_exp_and_others table -> zero table reloads.

Sharding: classes split across 8 cores (32 each); emb replicated.
"""

import numpy as np

B, D, C, S = 1024, 128, 256, 64
NCORES = 8
C_LOC = C // NCORES        # 32 classes per core
CS = C_LOC * S             # 2048 anchor rows per core
P = 128                    # partitions
NBT = B // P               # 8 batch tiles
NPAIR = C_LOC // 2         # 16 class pairs (128 anchor rows each)
EPS = 1e-8
SC_BIAS = 0.5 + EPS        # score = 0.5*cos + SC_BIAS
OUT_SCALE = 5.0            # ((1+x)/2 + 1e-8) / 0.1 = 5x + 5 + 1e-7
OUT_BIAS = 5.0 + 1e-7

_CACHE = {}


def _ones32() -> np.ndarray:
    """Block-ones stationary [128, NPAIR, C_LOC]: chunk q reduces partitions
    0:64 -> class 2q, 64:128 -> class 2q+1; all other columns zero so the 16
    matmuls can accumulate into one [C_LOC, ...] psum region."""
    o = np.zeros((P, NPAIR, C_LOC), np.float32)
    for q in range(NPAIR):
        o[0:64, q, 2 * q] = 1.0
        o[64:128, q, 2 * q + 1] = 1.0
    return o


def core_inputs(emb: np.ndarray, weight: np.ndarray, i: int) -> dict:
    """Host-side prep for core i: bf16 cast, V/W transposes, pair-packed
    Cholesky factors of the per-class Gram matrices."""
    import ml_dtypes

    bf = ml_dtypes.bfloat16
    Wc = np.ascontiguousarray(weight[i * C_LOC : (i + 1) * C_LOC]).astype(
        np.float64
    )                                              # [32, 64, 128]
    G = np.einsum("csd,ctd->cst", Wc, Wc)          # [32, 64, 64]
    jit = 1e-9 * np.einsum("css->c", G) / S
    G += jit[:, None, None] * np.eye(S)
    L = np.linalg.cholesky(G)                      # lower: G = L L^T
    lp = np.zeros((P, NPAIR, P), np.float32)
    for q in range(NPAIR):
        lp[0:S, q, 0:S] = L[2 * q]
        lp[S:P, q, S:P] = L[2 * q + 1]
    W2 = Wc.reshape(CS, D)
    nw = np.maximum(np.linalg.norm(W2, axis=1), EPS)
    V2 = W2 / nw[:, None]
    return {
        "emb": emb.astype(bf),
        "vt": np.ascontiguousarray(V2.T.astype(np.float32)).astype(bf),
        "wt": np.ascontiguousarray(W2.T.astype(np.float32)).astype(bf),
        "lp": np.ascontiguousarray(lp.reshape(P, NPAIR * P)).astype(bf),
        "ones32": np.ascontiguousarray(_ones32().reshape(P, NPAIR * C_LOC)).astype(bf),
    }


def build_nc():
    import concourse.bass as bass
    import concourse.tile as tile
    from concourse import bacc, mybir
    from concourse.masks import make_identity
    from contextlib import ExitStack

    f32 = mybir.dt.float32
    bf16 = mybir.dt.bfloat16
    AF = mybir.ActivationFunctionType
    ALU = mybir.AluOpType

    nc = bacc.Bacc(None, target_bir_lowering=False)
    emb_d = nc.dram_tensor("emb", [B, D], bf16, kind="ExternalInput")
    vt_d = nc.dram_tensor("vt", [D, CS], bf16, kind="ExternalInput")
    wt_d = nc.dram_tensor("wt", [D, CS], bf16, kind="ExternalInput")
    lp_d = nc.dram_tensor("lp", [P, NPAIR * P], bf16, kind="ExternalInput")
    o32_d = nc.dram_tensor("ones32", [P, NPAIR * C_LOC], bf16, kind="ExternalInput")
    out_d = nc.dram_tensor("out", [B, C_LOC], f32, kind="ExternalOutput")

    with tile.TileContext(nc) as tc, ExitStack() as ctx:
        sing = ctx.enter_context(tc.tile_pool(name="sing", bufs=1))
        dram = ctx.enter_context(tc.tile_pool(name="dram", bufs=1, space="DRAM"))
        wexp = ctx.enter_context(tc.tile_pool(name="wexp", bufs=4))
        wmask = ctx.enter_context(tc.tile_pool(name="wmask", bufs=3))
        wE = ctx.enter_context(tc.tile_pool(name="wE", bufs=2))
        wF = ctx.enter_context(tc.tile_pool(name="wF", bufs=3))
        wprod = ctx.enter_context(tc.tile_pool(name="wprod", bufs=3))
        small = ctx.enter_context(tc.tile_pool(name="small", bufs=2))
        ps_d = ctx.enter_context(tc.tile_pool(name="ps_d", bufs=2, space="PSUM"))
        ps_t = ctx.enter_context(tc.tile_pool(name="ps_t", bufs=2, space="PSUM"))
        ps_m = ctx.enter_context(tc.tile_pool(name="ps_m", bufs=2, space="PSUM"))
        ps_r = ctx.enter_context(tc.tile_pool(name="ps_r", bufs=1, space="PSUM"))
        ps_o = ctx.enter_context(tc.tile_pool(name="ps_o", bufs=1, space="PSUM"))

        # ---------------- input DMAs (two queues) ----------------
        En = sing.tile([P, NBT, D], bf16)
        nc.sync.dma_start(En[:], emb_d[:].rearrange("(t p) d -> p t d", p=P))
        VT = sing.tile([P, CS], bf16)
        nc.sync.dma_start(VT[:], vt_d[:])
        WT = sing.tile([P, CS], bf16)
        nc.gpsimd.dma_start(WT[:], wt_d[:])
        LP = sing.tile([P, NPAIR, P], bf16)
        nc.gpsimd.dma_start(LP[:], lp_d[:].rearrange("p (q m) -> p q m", q=NPAIR))
        O32 = sing.tile([P, NPAIR, C_LOC], bf16)
        nc.gpsimd.dma_start(O32[:], o32_d[:].rearrange("p (q m) -> p q m", q=NPAIR))

        ident = sing.tile([P, P], bf16)
        make_identity(nc, ident[:])
        sbias = sing.tile([P, 1], f32)
        nc.vector.memset(sbias[:], SC_BIAS)
        lhalf = sing.tile([P, 1], f32)
        nc.vector.memset(lhalf[:], float(np.log(0.5)))
        obias = sing.tile([P, 1], f32)
        nc.vector.memset(obias[:], OUT_BIAS)

        # warm the act table (ln+exp+square+copy+identity in one set)
        tdum = sing.tile([P, 1], f32)
        nc.scalar.activation(tdum[:], sbias[:], AF.Exp)
        nc.scalar.activation(tdum[:], sbias[:], AF.Ln)

        # ---------------- emb norms ----------------
        esq = sing.tile([P, NBT], f32)
        for t in range(NBT):
            j = small.tile([P, D], f32, tag="jact", bufs=2)
            nc.scalar.activation(j[:], En[:, t], AF.Square,
                                 accum_out=esq[:, t : t + 1])
        lesq = sing.tile([P, NBT], f32)
        nc.scalar.activation(lesq[:], esq[:], AF.Ln)
        ine = sing.tile([P, NBT], f32)       # 1/|emb| = exp(-0.5 ln esq)
        nc.scalar.activation(ine[:], lesq[:], AF.Exp, scale=-0.5)
        hine = sing.tile([P, NBT], f32)      # 0.5/|emb|
        nc.scalar.activation(hine[:], lesq[:], AF.Exp, scale=-0.5,
                             bias=lhalf[:])

        # ineT [C_LOC, B]: ine replicated across 32 partitions via DRAM bcast
        scr = dram.tile([B], f32)
        nc.sync.dma_start(scr[:].rearrange("(t p) -> p t", p=P), ine[:])
        scr_bc = bass.AP(
            tensor=scr[:].tensor, offset=scr[:].offset,
            ap=[[0, C_LOC]] + list(scr[:].ap),
        )
        ineT = sing.tile([C_LOC, B], f32)
        nc.sync.dma_start(ineT[:], scr_bc)

        # embT [D, B] bf16 via PE transposes
        embT = sing.tile([P, B], bf16)
        for g in range(2):
            pst = ps_t.tile([P, 4, P], bf16, tag="tr")
            for h in range(4):
                nc.tensor.transpose(pst[:, h], En[:, 4 * g + h], ident[:])
            nc.scalar.copy(embT[:, g * 512 : (g + 1) * 512],
                           pst[:].rearrange("p h x -> p (h x)"))

        # persistent output staging
        osb = sing.tile([P, NBT, C_LOC], f32)

        tiles = {}

        # ---------------- per-tile stages ----------------
        def SA(bt):
            """dot matmuls + exp"""
            bsl = slice(bt * P, (bt + 1) * P)
            exps = wexp.tile([P, CS], bf16, tag="exps")
            for j in range(4):
                js = slice(j * 512, (j + 1) * 512)
                psd = ps_d.tile([P, 512], f32, tag="dot")
                nc.tensor.matmul(psd[:], embT[:, bsl], VT[:, js])
                nc.scalar.activation(exps[:, js], psd[:], AF.Exp,
                                     bias=sbias[:], scale=hine[:, bt : bt + 1])
            tiles[("exps", bt)] = exps

        def SB(bt):
            """top-8 threshold per class + mask"""
            exps = tiles[("exps", bt)]
            m8 = small.tile([P, C_LOC, 8], bf16, tag="m8")
            for c in range(C_LOC):
                nc.vector.max(m8[:, c], exps[:, c * S : (c + 1) * S])
            mask = wmask.tile([P, CS], bf16, tag="mask")
            t8b = m8[:, :, 7:8].to_broadcast([P, C_LOC, S])
            nc.vector.tensor_tensor(
                mask[:].rearrange("p (c s) -> p c s", c=C_LOC),
                exps[:].rearrange("p (c s) -> p c s", c=C_LOC),
                t8b, op=ALU.is_ge)
            tiles[("mask", bt)] = mask

        def SC(bt):
            """masked E + transposes to s-major F"""
            exps = tiles.pop(("exps", bt))
            mask = tiles.pop(("mask", bt))
            E = wE.tile([P, CS], bf16, tag="E")
            nc.gpsimd.tensor_mul(E[:], mask[:], exps[:])
            F = wF.tile([P, CS], bf16, tag="F")
            for g in range(4):
                pst = ps_t.tile([P, 4, P], bf16, tag="tr")
                for h in range(4):
                    q = 4 * g + h
                    nc.tensor.transpose(pst[:, h], E[:, q * P : (q + 1) * P],
                                        ident[:])
                nc.scalar.copy(F[:, g * 512 : (g + 1) * 512],
                               pst[:].rearrange("p h x -> p (h x)"))
            tiles[("F", bt)] = F

        def SD(bt):
            """M = LP^T F -> Msq; dotT = W embT -> prodD"""
            bsl = slice(bt * P, (bt + 1) * P)
            F = tiles[("F", bt)]
            prod2 = wprod.tile([P, NPAIR, 2, P], bf16, tag="p2")
            for g in range(4):
                F3 = F[:, g * 512 : (g + 1) * 512].rearrange(
                    "p (h x) -> p h x", h=4)
                psm = ps_m.tile([P, 4, P], f32, tag="mm")
                for h in range(4):
                    q = 4 * g + h
                    nc.tensor.matmul(psm[:, h], LP[:, q],
                                     F[:, q * P : (q + 1) * P])
                nc.scalar.activation(prod2[:, 4 * g : 4 * g + 4, 0, :],
                                     psm[:], AF.Square)
                psw = ps_m.tile([P, 4, P], f32, tag="mm")
                for h in range(4):
                    q = 4 * g + h
                    nc.tensor.matmul(psw[:, h], WT[:, q * P : (q + 1) * P],
                                     embT[:, bsl])
                dsb = wmask.tile([P, 4, P], bf16, tag="dsb")
                nc.scalar.copy(dsb[:], psw[:])
                nc.gpsimd.tensor_mul(prod2[:, 4 * g : 4 * g + 4, 1, :],
                                     dsb[:], F3)
            tiles[("p2", bt)] = prod2

        def SE(bt):
            """segmented s-sums via accumulating block-ones matmuls"""
            prod2 = tiles.pop(("p2", bt))
            psr = ps_r.tile([C_LOC, 2, P], f32, tag="red")
            for q in range(NPAIR):
                nc.tensor.matmul(psr[:], O32[:, q], prod2[:, q],
                                 start=(q == 0), stop=(q == NPAIR - 1))
            tiles[("red", bt)] = psr

        def SF(bt):
            """tail: rsqrt via ln/exp, cosine, scale, transpose out"""
            psr = tiles.pop(("red", bt))
            lnp = small.tile([C_LOC, P], f32, tag="lnp")
            nc.scalar.activation(lnp[:], psr[:, 0], AF.Ln)
            rnp = small.tile([C_LOC, P], f32, tag="rnp")
            nc.scalar.activation(rnp[:], lnp[:], AF.Exp, scale=-0.5)
            c1 = small.tile([C_LOC, P], bf16, tag="c1")
            nc.vector.tensor_mul(c1[:], psr[:, 1], rnp[:])
            c2 = small.tile([C_LOC, P], bf16, tag="c2")
            nc.vector.tensor_mul(c2[:], c1[:], ineT[:, bt * P : (bt + 1) * P])
            pso = ps_o.tile([P, C_LOC], bf16, tag="out")
            nc.tensor.transpose(pso[:], c2[:], ident[:C_LOC, :C_LOC])
            nc.scalar.activation(osb[:, bt], pso[:], AF.Identity,
                                 bias=obias[:], scale=OUT_SCALE)

        # ---------------- software-pipelined loop ----------------
        for r in range(NBT + 5):
            if 0 <= r - 5:
                SF(r - 5)
            if 0 <= r - 4 < NBT:
                SE(r - 4)
            if 0 <= r - 3 < NBT:
                SD(r - 3)
            if 0 <= r - 2 < NBT:
                SC(r - 2)
            if 0 <= r - 1 < NBT:
                SB(r - 1)
            if r < NBT:
                SA(r)

        nc.sync.dma_start(out_d[:].rearrange("(t p) c -> p t c", p=P), osb[:])

    nc.compile()
    return nc


def _get_nc():
    if "nc" not in _CACHE:
        _CACHE["nc"] = build_nc()
    return _CACHE["nc"]


def kernel(emb: np.ndarray, weight: np.ndarray) -> np.ndarray:
    from concourse.bass_utils import run_bass_kernel_spmd

    emb = np.ascontiguousarray(np.asarray(emb, dtype=np.float32))
    weight = np.ascontiguousarray(np.asarray(weight, dtype=np.float32))
    assert emb.shape == (B, D) and weight.shape == (C, S, D)

    nc = _get_nc()
    in_maps = [core_inputs(emb, weight, i) for i in range(NCORES)]
    res = run_bass_kernel_spmd(nc, in_maps, core_ids=list(range(NCORES)))
    return np.concatenate(
        [res.results[i]["out"] for i in range(NCORES)], axis=1
    ).astype(np.float32)
